# revision 21
# baseline (speedup 1.0000x reference)
"""Distributed kNN classifier for Trainium2 (8 NeuronCores).

Strategy
--------
reference(...) computes sim = feature @ feature_bank  [B, N], takes top-k
(k=200) per query, exp(sim/0.1) weights, scatter-adds into per-class scores
and returns the descending stable argsort of those scores.

The heavy part is the [1024, 1024] @ [1024, 100000] matmul plus top-k.
feature_bank is sharded along N across the 8 cores (12500 cols each).

Device (default, fp8): each core computes its sim shard with an fp8e4m3
DoubleRow matmul (fp32 PSUM accumulation, 2 MACs/cell/cycle) and writes
uint8 `clamp(round(sim - 64), 0, 255)` — candidate mask and coarse value in
one byte.  Sims are ~N(0, 32^2); every query's true 200th-largest sim is
>= ~84, and the fp8 matmul error is bounded by E_FP8, so the candidates
with stored value >= 10 (sim >~ 74) are a guaranteed superset of the true
top-k.  The host then (a) keeps, per query, only candidates
within 2*E of the device-value 200th-largest (a confidence window that
provably contains the true top-k), (b) recomputes exact fp32 similarities
for those ~0.4% of pairs, (c) selects the exact top-k with jax.lax.top_k
tie semantics and replicates the reference's exp/scatter/argsort in numpy.
If any query yields fewer than k candidates, the host falls back to an
exact full-row recompute for it, so correctness never depends on the
threshold.

A bf16 variant (KNN_IMPL=bf16) with a uint8 `sim > T0` mask output is kept
as a fallback.
"""

import os
import sys
import time
import numpy as np
import ml_dtypes


def _tlog(msg, _t=[None]):
    if os.environ.get("KNN_TIMING"):
        now = time.time()
        dt = 0.0 if _t[0] is None else now - _t[0]
        _t[0] = now
        print(f"[knn +{dt:6.2f}s] {msg}", file=sys.stderr, flush=True)


import concourse.bass as bass
import concourse.bacc as bacc
import concourse.mybir as mybir
from concourse import tile
from concourse.bass_utils import run_bass_kernel_spmd

# Problem geometry (hardcoded per spec).
B = 1024          # queries
D = 1024          # feature dim
N_TOTAL = 100000  # bank size
N_CORES = 8
N_SHARD = N_TOTAL // N_CORES  # 12500

P = 128           # partitions
KCH = D // P      # 8 contraction chunks (bf16)
KK = D // (2 * P)  # 4 double-row contraction chunks (fp8)
QW = 512          # rhs free width per matmul (one PSUM bank of fp32)
CH = 512          # bank columns loaded per DMA chunk

T0 = 80.0         # bf16 mask threshold (true 200th-largest sim is >= ~84.2)
VAL_OFF = 64.0    # u8 value-output offset: stored = clamp(sim - 64, 0, 255)
T0_FP8_U8 = 10    # u8 threshold (sim >~ 74; fp8 |err| <= ~6.6, margin ~10)
E_FP8 = 8.5       # fp8 matmul + u8 quantization error bound for the window

KNN_T = 0.1

LAST_EXEC_TIME_NS = None
LAST_DEV_VALS = None  # [N, B] bf16 device sims (fp8 path), for diagnostics


def _build_program_bf16(n_shard: int = N_SHARD):
    """bf16 matmul; uint8 mask output."""
    nc = bacc.Bacc("TRN2", target_bir_lowering=False, debug=False)

    featT = nc.dram_tensor("featT", [D, B], mybir.dt.bfloat16, kind="ExternalInput")
    bank = nc.dram_tensor("bank", [D, n_shard], mybir.dt.bfloat16, kind="ExternalInput")
    mask = nc.dram_tensor("mask", [n_shard, B], mybir.dt.uint8, kind="ExternalOutput")

    with tile.TileContext(nc) as tc:
        with (
            tc.tile_pool(name="feat", bufs=1) as feat_pool,
            tc.tile_pool(name="bankp", bufs=4) as bank_pool,
            tc.tile_pool(name="maskp", bufs=6) as mask_pool,
            tc.tile_pool(name="psum", bufs=6, space=bass.MemorySpace.PSUM) as psum_pool,
        ):
            # All of feature^T stays resident: [128, 8, 1024] bf16 (16 KiB/part)
            featT_sb = feat_pool.tile([P, KCH, B], mybir.dt.bfloat16)
            for kc in range(KCH):
                nc.sync.dma_start(featT_sb[:, kc, :], featT[kc * P:(kc + 1) * P, :])

            nch = (n_shard + CH - 1) // CH
            for ci in range(nch):
                c0 = ci * CH
                cw = min(CH, n_shard - c0)
                bank_sb = bank_pool.tile([P, KCH, CH], mybir.dt.bfloat16)
                for kc in range(KCH):
                    nc.sync.dma_start(
                        bank_sb[:, kc, :cw], bank[kc * P:(kc + 1) * P, c0:c0 + cw]
                    )
                for si in range(0, cw, P):
                    sw = min(P, cw - si)
                    mask_t = mask_pool.tile([P, B], mybir.dt.uint8)
                    for qh in range(B // QW):
                        ps = psum_pool.tile([P, QW], mybir.dt.float32)
                        for kc in range(KCH):
                            nc.tensor.matmul(
                                ps[:sw, :],
                                bank_sb[:, kc, si:si + sw],
                                featT_sb[:, kc, qh * QW:(qh + 1) * QW],
                                start=(kc == 0),
                                stop=(kc == KCH - 1),
                            )
                        nc.vector.tensor_scalar(
                            out=mask_t[:sw, qh * QW:(qh + 1) * QW],
                            in0=ps[:sw, :],
                            scalar1=T0,
                            scalar2=None,
                            op0=mybir.AluOpType.is_gt,
                        )
                    nc.sync.dma_start(mask[c0 + si:c0 + si + sw, :], mask_t[:sw, :])

    nc.compile()
    return nc


N_PAD = 12512  # n_shard padded so every chunk width is a multiple of 16
# Graduated chunk widths (bank cols per DMA): small first chunk so the first
# MM group starts ASAP after featT lands; small last chunks so the final
# drain+DMA tail after the last matmul is short.  Multiples of 128 except the
# tail (96), so slices stay full-width: 98 slices = 784 matmuls total.
CHUNKS = [128, 128, 256, 512] + [1024] * 11 + [128, 96]
assert sum(CHUNKS) == N_PAD
W_WARM = 18  # junk warm-up matmuls that run while input DMAs are in flight


def _build_program_fp8(n_shard: int = N_SHARD):
    """fp8e4m3 DoubleRow matmul (K=256 per instruction).

    Output: uint8 `clamp(sim - VAL_OFF, 0, 255)` — mask and coarse value in
    one byte (sims of interest are in [74, ~155], so VAL_OFF=64 maps them to
    [10, ~91]; anything below 64 clamps to 0 via the fused max).

    v2 layout: inputs arrive pre-tiled from the host —
      featT8 [128, KK, 2, B]   (one fully contiguous 8 KiB/partition DMA)
      bank8  [128, 8, N_PAD]   (plane j = row j*128+p of the original [D, n]
                                shard; one DMA instruction per column chunk)
    A burst of junk warm-up matmuls keeps the PE busy during the input DMA
    wait so the HAM clock-gate ramp (K=4/8 -> 8/8) happens off the critical
    path and the real matmuls start at full rate.
    """
    assert n_shard == N_SHARD
    nc = bacc.Bacc("TRN2", target_bir_lowering=bool(int(os.environ.get("KNN_BIRLOWER", "0"))), debug=False)

    # featT8 is stored as two contiguous query-halves so the first matmuls
    # can start after only half of feature^T has landed.
    featT8 = nc.dram_tensor("featT8", [P, 2, KK, 2, QW], mybir.dt.float8e4, kind="ExternalInput")
    bank8 = nc.dram_tensor("bank8", [P, 2 * KK, N_PAD], mybir.dt.float8e4, kind="ExternalInput")
    vals = nc.dram_tensor("vals", [N_PAD, B], mybir.dt.uint8, kind="ExternalOutput")

    n_warm = int(os.environ.get("KNN_WARM", str(W_WARM)))
    with tile.TileContext(nc) as tc:
        with (
            tc.tile_pool(name="warm", bufs=1) as warm_pool,
            tc.tile_pool(name="feat", bufs=1) as feat_pool,
            tc.tile_pool(name="bankp", bufs=6) as bank_pool,
            tc.tile_pool(name="valp", bufs=6) as val_pool,
            tc.tile_pool(name="psum", bufs=3, space=bass.MemorySpace.PSUM) as psum_pool,
        ):
            # Warm-up: junk DoubleRow matmuls on a zeroed tile.  No input
            # deps, so they run during the featT/bank DMA wait at the head.
            if n_warm:
                warm_sb = warm_pool.tile([P, 2, QW], mybir.dt.float8e4)
                nc.gpsimd.memset(warm_sb[:, :, :], 0)
                for _ in range(n_warm):
                    # Shares the "ps_h" PSUM tag (1 bank x 2 bufs) with the
                    # final slice's half-tiles: 3x2 + 2x1 = 8 banks total.
                    wps = psum_pool.tile([P, QW], mybir.dt.float32, name="ps_h", bufs=2)
                    nc.tensor.matmul(
                        wps[:, :],
                        warm_sb[:, :, :P],
                        warm_sb[:, :, :],
                        start=True,
                        stop=True,
                        perf_mode=mybir.MatmulPerfMode.DoubleRow,
                    )

            # Head DMA order (all on the Sync queue — secondary engine
            # queues get far worse DMA service): chunk0 (small) first, then
            # the two featT halves.  The first matmul group needs only
            # chunk0 + featT half 0.
            bank_sb0 = bank_pool.tile([P, 2 * KK, CHUNKS[0]], mybir.dt.float8e4)
            nc.sync.dma_start(bank_sb0[:, :, :], bank8[:, :, 0:CHUNKS[0]])

            # feature^T resident as two query-half tiles [128, kk, 2, QW] fp8
            # (4 KiB/part each); separate tiles so the qh=0 matmuls depend
            # only on the first half's DMA.  The middle "2" dim is the
            # DoubleRow K-pair (two 128-row planes of D).
            featT_h = []
            for h in range(2):
                f = feat_pool.tile([P, KK, 2, QW], mybir.dt.float8e4, name=f"featT_h{h}")
                nc.sync.dma_start(f[:, :, :, :], featT8[:, h, :, :, :])
                featT_h.append(f)

            def mm_group(ps, bank_sb, si, sw, qh):
                for kk in range(KK):
                    nc.tensor.matmul(
                        ps[:sw, qh * QW:(qh + 1) * QW],
                        bank_sb[:, 2 * kk:2 * kk + 2, si:si + sw],
                        featT_h[qh][:, kk, :, :],
                        start=(kk == 0),
                        stop=(kk == KK - 1),
                        perf_mode=mybir.MatmulPerfMode.DoubleRow,
                    )

            def drain(ps, val_t, sw, c0, si):
                # (sim - 64) clamped at 0, cast to u8 — single DVE op.
                nc.vector.tensor_scalar(
                    out=val_t[:sw, :],
                    in0=ps[:sw, :],
                    scalar1=-VAL_OFF,
                    scalar2=0.0,
                    op0=mybir.AluOpType.add,
                    op1=mybir.AluOpType.max,
                )
                nc.sync.dma_start(vals[c0 + si:c0 + si + sw, :], val_t[:sw, :])

            c0 = 0
            for ci, cw in enumerate(CHUNKS):
                last = ci == len(CHUNKS) - 1
                if ci == 0:
                    bank_sb = bank_sb0
                else:
                    bank_sb = bank_pool.tile([P, 2 * KK, cw], mybir.dt.float8e4, name="bank_sb")
                    nc.sync.dma_start(bank_sb[:, :, :], bank8[:, :, c0:c0 + cw])
                for si in range(0, cw, P):
                    sw = min(P, cw - si)
                    ps = psum_pool.tile([P, B], mybir.dt.float32, name="ps")
                    for qh in range(B // QW):
                        mm_group(ps, bank_sb, si, sw, qh)
                    if last and si + P >= cw:
                        # Final slice: drain and store per query-half so the
                        # tail after the very last matmul is short.
                        for qh in range(B // QW):
                            val_h = val_pool.tile([P, QW], mybir.dt.uint8)
                            nc.vector.tensor_scalar(
                                out=val_h[:sw, :],
                                in0=ps[:sw, qh * QW:(qh + 1) * QW],
                                scalar1=-VAL_OFF,
                                scalar2=0.0,
                                op0=mybir.AluOpType.add,
                                op1=mybir.AluOpType.max,
                            )
                            nc.sync.dma_start(
                                vals[c0 + si:c0 + si + sw, qh * QW:(qh + 1) * QW],
                                val_h[:sw, :],
                            )
                    else:
                        val_t = val_pool.tile([P, B], mybir.dt.uint8)
                        drain(ps, val_t, sw, c0, si)
                c0 += cw

    nc.compile()
    return nc


_PROGRAM_CACHE = {}


def _get_program(impl, n_shard):
    key = (impl, n_shard)
    if key not in _PROGRAM_CACHE:
        build = _build_program_fp8 if impl == "fp8" else _build_program_bf16
        _PROGRAM_CACHE[key] = build(n_shard)
    return _PROGRAM_CACHE[key]


def _profile_ctx():
    import contextlib

    @contextlib.contextmanager
    def _maybe_profile():
        """Optional NTFF capture via the axon NRT-profile C ABI."""
        prof_dir = os.environ.get("KNN_PROFILE_DIR")
        if not prof_dir:
            yield
            return
        import ctypes
        lib = ctypes.CDLL("/opt/axon/libaxon_pjrt.so")
        lib.axon_start_nrt_profile.argtypes = [
            ctypes.POINTER(ctypes.c_int64), ctypes.c_size_t]
        lib.axon_start_nrt_profile.restype = ctypes.c_int64
        lib.axon_stop_nrt_profile.argtypes = [ctypes.c_char_p]
        lib.axon_stop_nrt_profile.restype = ctypes.c_int64
        import jax
        jax.devices()
        rc = lib.axon_start_nrt_profile(None, 0)
        if rc != 0:
            raise RuntimeError(f"axon_start_nrt_profile rc={rc}")
        try:
            yield
        finally:
            n = lib.axon_stop_nrt_profile(str(prof_dir).encode())
            print(f"ntff profile: {n} file(s) -> {prof_dir}", flush=True)

    return _maybe_profile()


def _run_spmd(nc, in_maps):
    global LAST_EXEC_TIME_NS
    with _profile_ctx():
        res = run_bass_kernel_spmd(
            nc, in_maps, core_ids=list(range(N_CORES)), trace=False
        )
    LAST_EXEC_TIME_NS = res.exec_time_ns
    _tlog("device run done")
    return res


def _candidate_pairs_bf16(feature, bank_f32):
    """bf16+mask path: device mask -> all candidate pairs."""
    n = bank_f32.shape[1]
    n_shard = n // N_CORES
    nc = _get_program("bf16", n_shard)
    _tlog("program built")

    featT_bf = np.ascontiguousarray(feature.T).astype(ml_dtypes.bfloat16)
    bank_bf = bank_f32.astype(ml_dtypes.bfloat16)
    in_maps = [
        {
            "featT": featT_bf,
            "bank": np.ascontiguousarray(bank_bf[:, i * n_shard:(i + 1) * n_shard]),
        }
        for i in range(N_CORES)
    ]
    res = _run_spmd(nc, in_maps)
    mask = np.concatenate([res.results[i]["mask"] for i in range(N_CORES)], axis=0)

    nidx, qidx = np.nonzero(mask)  # [N, B]: sorted by bank idx
    order = np.argsort(qidx, kind="stable")  # per-query segments, nidx ascending
    qidx = qidx[order]
    nidx = nidx[order]
    counts = np.bincount(qidx, minlength=feature.shape[0])
    starts = np.zeros(feature.shape[0] + 1, dtype=np.int64)
    np.cumsum(counts, out=starts[1:])
    _tlog(f"candidates built ({len(nidx)} pairs)")
    return qidx, nidx, starts


def _candidate_pairs_fp8(feature, bank_f32, k):
    """fp8+values path: threshold, then keep only the top-k confidence window."""
    global LAST_DEV_VALS
    n = bank_f32.shape[1]
    n_shard = n // N_CORES
    nc = _get_program("fp8", n_shard)
    _tlog("program built")

    # Pre-tile to the device layouts (see _build_program_fp8 docstring).
    featT_8 = np.ascontiguousarray(feature.T).astype(ml_dtypes.float8_e4m3)
    featT_t = (
        featT_8.reshape(KK, 2, P, B).transpose(2, 0, 1, 3)
    )  # [128, KK, 2, B]; plane (kk,i) holds rows (2kk+i)*128+p
    featT_t = np.ascontiguousarray(
        featT_t.reshape(P, KK, 2, 2, QW).transpose(0, 3, 1, 2, 4)
    )  # [128, 2(q-half), KK, 2, QW] — each query-half contiguous
    bank_8 = bank_f32.astype(ml_dtypes.float8_e4m3)
    bank_t = bank_8.reshape(2 * KK, P, n).transpose(1, 0, 2)  # [128, 8, n]
    pad = np.zeros((P, 2 * KK, N_PAD - n_shard), dtype=ml_dtypes.float8_e4m3)
    in_maps = [
        {
            "featT8": featT_t,
            "bank8": np.ascontiguousarray(
                np.concatenate(
                    [bank_t[:, :, i * n_shard:(i + 1) * n_shard], pad], axis=2
                )
            ),
        }
        for i in range(N_CORES)
    ]
    res = _run_spmd(nc, in_maps)
    vals = np.concatenate(
        [res.results[i]["vals"][:n_shard] for i in range(N_CORES)], axis=0
    )
    LAST_DEV_VALS = vals  # [N, B] u8: clamp(sim - VAL_OFF, 0, 255)

    m = vals >= np.uint8(T0_FP8_U8)
    nidx, qidx = np.nonzero(m)
    # The DVE f32->u8 cast rounds-to-nearest, so stored+VAL_OFF is already the
    # quantization-interval midpoint (E_FP8 covers the +-0.5 either way).
    dv = vals[nidx, qidx].astype(np.float32) + np.float32(VAL_OFF)
    order = np.argsort(qidx, kind="stable")  # per-query segments, nidx ascending
    qidx = qidx[order]
    nidx = nidx[order]
    dv = dv[order]
    b = feature.shape[0]
    counts = np.bincount(qidx, minlength=b)
    starts_all = np.zeros(b + 1, dtype=np.int64)
    np.cumsum(counts, out=starts_all[1:])
    _tlog(f"thresholded ({len(nidx)} pairs)")

    # Per query, keep only candidates that can possibly be in the true top-k:
    # dev >= dev_rank_k - 2E (see module docstring for the bound).
    keep = np.zeros(len(nidx), dtype=bool)
    for q in range(b):
        s, e = starts_all[q], starts_all[q + 1]
        c = e - s
        if c < k:
            keep[s:e] = True  # top-k loop will take the full-row fallback
            continue
        seg = dv[s:e]
        rk = np.partition(seg, c - k)[c - k]
        keep[s:e] = seg >= rk - 2.0 * E_FP8
    qidx = qidx[keep]
    nidx = nidx[keep]
    counts = np.bincount(qidx, minlength=b)
    starts = np.zeros(b + 1, dtype=np.int64)
    np.cumsum(counts, out=starts[1:])
    _tlog(f"windowed ({len(nidx)} pairs)")
    return qidx, nidx, starts


def _finish(feature, bank_f32, labels, num_classes, k, cand):
    """Exact fp32 re-rank of candidate pairs + reference post-processing.

    cand is (qidx, nidx, starts) or None (full host fallback).
    """
    b, d = feature.shape
    n = bank_f32.shape[1]

    if cand is not None:
        qidx, nidx, starts = cand
        bankT = np.ascontiguousarray(bank_f32.T)  # contiguous row gathers
        _tlog("bankT transpose done")
        vals = np.empty(len(nidx), dtype=np.float32)
        CHP = 1 << 16
        for s in range(0, len(nidx), CHP):
            e = min(s + CHP, len(nidx))
            vals[s:e] = np.einsum(
                "ij,ij->i", feature[qidx[s:e]], bankT[nidx[s:e]]
            )
        _tlog(f"exact vals done ({len(nidx)} pairs)")

    full_rows = None
    full_q0 = 0
    all_idx = np.arange(n)

    sel_q = np.empty(b * k, dtype=np.int64)
    sel_lab = np.empty(b * k, dtype=np.int64)
    sel_val = np.empty(b * k, dtype=np.float32)
    pos = 0
    ROWBLK = 64
    for q in range(b):
        if cand is not None and starts[q + 1] - starts[q] >= k:
            s, e = starts[q], starts[q + 1]
            v = vals[s:e]
            idx = nidx[s:e]
        else:
            # Exact full row (no device pre-filter, or threshold miss).
            if full_rows is None or not (full_q0 <= q < full_q0 + ROWBLK):
                full_q0 = q
                hi = min(q + ROWBLK, b)
                full_rows = feature[q:hi] @ bank_f32
            v = full_rows[q - full_q0]
            idx = all_idx
        # jax.lax.top_k semantics: descending, ties -> lower index first.
        order = np.argsort(-v, kind="stable")[:k]
        sel_q[pos:pos + k] = q
        sel_lab[pos:pos + k] = labels[idx[order]]
        sel_val[pos:pos + k] = v[order]
        pos += k
    _tlog("per-query topk done")

    with np.errstate(over="ignore"):
        w = np.exp(sel_val / np.float32(KNN_T)).astype(np.float32)
    scores = np.zeros((b, num_classes), dtype=np.float32)
    np.add.at(scores, (sel_q, sel_lab), w)
    _tlog("scatter done")
    return scores


def kernel(feature, feature_bank, feature_labels, num_classes, knn_k):
    _tlog("kernel() start")
    feature = np.asarray(feature, dtype=np.float32)
    bank_f32 = np.asarray(feature_bank, dtype=np.float32)
    labels = np.asarray(feature_labels)
    c = int(np.asarray(num_classes))
    k = int(np.asarray(knn_k))

    b, d = feature.shape
    n = bank_f32.shape[1]

    impl = os.environ.get("KNN_IMPL", "fp8")
    use_device = d == D and b == B and n % N_CORES == 0 and n // N_CORES > 0
    if use_device:
        if impl == "fp8":
            cand = _candidate_pairs_fp8(feature, bank_f32, k)
        else:
            cand = _candidate_pairs_bf16(feature, bank_f32)
    else:
        cand = None  # degenerate fallback: host does it all

    scores = _finish(feature, bank_f32, labels, c, k, cand)
    pred = np.argsort(-scores, axis=1, kind="stable").astype(np.int32)
    _tlog("final argsort done")
    return pred



# revision 23
# speedup vs baseline: 1.0049x; 1.0049x over previous
"""Distributed kNN classifier for Trainium2 (8 NeuronCores).

Strategy
--------
reference(...) computes sim = feature @ feature_bank  [B, N], takes top-k
(k=200) per query, exp(sim/0.1) weights, scatter-adds into per-class scores
and returns the descending stable argsort of those scores.

The heavy part is the [1024, 1024] @ [1024, 100000] matmul plus top-k.
feature_bank is sharded along N across the 8 cores (12500 cols each).

Device (default, fp8): each core computes its sim shard with an fp8e4m3
DoubleRow matmul (fp32 PSUM accumulation, 2 MACs/cell/cycle) and writes
uint8 `clamp(round(sim - 64), 0, 255)` — candidate mask and coarse value in
one byte.  Sims are ~N(0, 32^2); every query's true 200th-largest sim is
>= ~84, and the fp8 matmul error is bounded by E_FP8, so the candidates
with stored value >= 10 (sim >~ 74) are a guaranteed superset of the true
top-k.  The host then (a) keeps, per query, only candidates
within 2*E of the device-value 200th-largest (a confidence window that
provably contains the true top-k), (b) recomputes exact fp32 similarities
for those ~0.4% of pairs, (c) selects the exact top-k with jax.lax.top_k
tie semantics and replicates the reference's exp/scatter/argsort in numpy.
If any query yields fewer than k candidates, the host falls back to an
exact full-row recompute for it, so correctness never depends on the
threshold.

A bf16 variant (KNN_IMPL=bf16) with a uint8 `sim > T0` mask output is kept
as a fallback.
"""

import os
import sys
import time
import numpy as np
import ml_dtypes


def _tlog(msg, _t=[None]):
    if os.environ.get("KNN_TIMING"):
        now = time.time()
        dt = 0.0 if _t[0] is None else now - _t[0]
        _t[0] = now
        print(f"[knn +{dt:6.2f}s] {msg}", file=sys.stderr, flush=True)


import concourse.bass as bass
import concourse.bacc as bacc
import concourse.mybir as mybir
from concourse import tile
from concourse.bass_utils import run_bass_kernel_spmd

# Problem geometry (hardcoded per spec).
B = 1024          # queries
D = 1024          # feature dim
N_TOTAL = 100000  # bank size
N_CORES = 8
N_SHARD = N_TOTAL // N_CORES  # 12500

P = 128           # partitions
KCH = D // P      # 8 contraction chunks (bf16)
KK = D // (2 * P)  # 4 double-row contraction chunks (fp8)
QW = 512          # rhs free width per matmul (one PSUM bank of fp32)
CH = 512          # bank columns loaded per DMA chunk

T0 = 80.0         # bf16 mask threshold (true 200th-largest sim is >= ~84.2)
VAL_OFF = 64.0    # u8 value-output offset: stored = clamp(sim - 64, 0, 255)
T0_FP8_U8 = 10    # u8 threshold (sim >~ 74; fp8 |err| <= ~6.6, margin ~10)
E_FP8 = 8.5       # fp8 matmul + u8 quantization error bound for the window

KNN_T = 0.1

LAST_EXEC_TIME_NS = None
LAST_DEV_VALS = None  # [N, B] bf16 device sims (fp8 path), for diagnostics


def _build_program_bf16(n_shard: int = N_SHARD):
    """bf16 matmul; uint8 mask output."""
    nc = bacc.Bacc("TRN2", target_bir_lowering=False, debug=False)

    featT = nc.dram_tensor("featT", [D, B], mybir.dt.bfloat16, kind="ExternalInput")
    bank = nc.dram_tensor("bank", [D, n_shard], mybir.dt.bfloat16, kind="ExternalInput")
    mask = nc.dram_tensor("mask", [n_shard, B], mybir.dt.uint8, kind="ExternalOutput")

    with tile.TileContext(nc) as tc:
        with (
            tc.tile_pool(name="feat", bufs=1) as feat_pool,
            tc.tile_pool(name="bankp", bufs=4) as bank_pool,
            tc.tile_pool(name="maskp", bufs=6) as mask_pool,
            tc.tile_pool(name="psum", bufs=6, space=bass.MemorySpace.PSUM) as psum_pool,
        ):
            # All of feature^T stays resident: [128, 8, 1024] bf16 (16 KiB/part)
            featT_sb = feat_pool.tile([P, KCH, B], mybir.dt.bfloat16)
            for kc in range(KCH):
                nc.sync.dma_start(featT_sb[:, kc, :], featT[kc * P:(kc + 1) * P, :])

            nch = (n_shard + CH - 1) // CH
            for ci in range(nch):
                c0 = ci * CH
                cw = min(CH, n_shard - c0)
                bank_sb = bank_pool.tile([P, KCH, CH], mybir.dt.bfloat16)
                for kc in range(KCH):
                    nc.sync.dma_start(
                        bank_sb[:, kc, :cw], bank[kc * P:(kc + 1) * P, c0:c0 + cw]
                    )
                for si in range(0, cw, P):
                    sw = min(P, cw - si)
                    mask_t = mask_pool.tile([P, B], mybir.dt.uint8)
                    for qh in range(B // QW):
                        ps = psum_pool.tile([P, QW], mybir.dt.float32)
                        for kc in range(KCH):
                            nc.tensor.matmul(
                                ps[:sw, :],
                                bank_sb[:, kc, si:si + sw],
                                featT_sb[:, kc, qh * QW:(qh + 1) * QW],
                                start=(kc == 0),
                                stop=(kc == KCH - 1),
                            )
                        nc.vector.tensor_scalar(
                            out=mask_t[:sw, qh * QW:(qh + 1) * QW],
                            in0=ps[:sw, :],
                            scalar1=T0,
                            scalar2=None,
                            op0=mybir.AluOpType.is_gt,
                        )
                    nc.sync.dma_start(mask[c0 + si:c0 + si + sw, :], mask_t[:sw, :])

    nc.compile()
    return nc


N_PAD = 12512  # n_shard padded so every chunk width is a multiple of 16
# Graduated chunk widths (bank cols per DMA): small first chunk so the first
# MM group starts ASAP after featT lands; small last chunks so the final
# drain+DMA tail after the last matmul is short.  Multiples of 128 except the
# tail (96), so slices stay full-width: 98 slices = 784 matmuls total.
CHUNKS = [128, 128, 256, 512] + [1024] * 11 + [128, 96]
assert sum(CHUNKS) == N_PAD
W_WARM = 18  # junk warm-up matmuls that run while input DMAs are in flight


def _build_program_fp8(n_shard: int = N_SHARD):
    """fp8e4m3 DoubleRow matmul (K=256 per instruction).

    Output: uint8 `clamp(sim - VAL_OFF, 0, 255)` — mask and coarse value in
    one byte (sims of interest are in [74, ~155], so VAL_OFF=64 maps them to
    [10, ~91]; anything below 64 clamps to 0 via the fused max).

    v2 layout: inputs arrive pre-tiled from the host —
      featT8 [128, 2, KK, 2, QW]  (two contiguous 4 KiB/partition DMAs, one
                                   per query-half, so qh=0 matmuls start
                                   after only half of feature^T has landed)
      bank8  [128, 8, N_PAD]      (plane j = row j*128+p of the original
                                   [D, n] shard; one DMA per column chunk)
    A burst of junk warm-up matmuls keeps the PE busy during the input DMA
    wait so the HAM clock-gate ramp (K=4/8 -> 8/8) happens off the critical
    path and the real matmuls start at full rate; the bridge gap to the
    first real matmul must stay under the ~1.7us warm MID window or the PE
    re-throttles.
    """
    assert n_shard == N_SHARD
    nc = bacc.Bacc("TRN2", target_bir_lowering=False, debug=False)

    # featT8 is stored as two contiguous query-halves so the first matmuls
    # can start after only half of feature^T has landed.
    featT8 = nc.dram_tensor("featT8", [P, 2, KK, 2, QW], mybir.dt.float8e4, kind="ExternalInput")
    bank8 = nc.dram_tensor("bank8", [P, 2 * KK, N_PAD], mybir.dt.float8e4, kind="ExternalInput")
    vals = nc.dram_tensor("vals", [N_PAD, B], mybir.dt.uint8, kind="ExternalOutput")

    n_warm = int(os.environ.get("KNN_WARM", str(W_WARM)))
    with tile.TileContext(nc) as tc:
        with (
            tc.tile_pool(name="warm", bufs=1) as warm_pool,
            tc.tile_pool(name="feat", bufs=1) as feat_pool,
            tc.tile_pool(name="bankp", bufs=6) as bank_pool,
            tc.tile_pool(name="valp", bufs=6) as val_pool,
            tc.tile_pool(name="psum", bufs=3, space=bass.MemorySpace.PSUM) as psum_pool,
        ):
            # Warm-up: junk DoubleRow matmuls on a zeroed tile.  No input
            # deps, so they run during the featT/bank DMA wait at the head.
            if n_warm:
                warm_sb = warm_pool.tile([P, 2, QW], mybir.dt.float8e4)
                nc.gpsimd.memset(warm_sb[:, :, :], 0)
                for _ in range(n_warm):
                    # Shares the "ps_h" PSUM tag (1 bank x 2 bufs) with the
                    # final slice's half-tiles: 3x2 + 2x1 = 8 banks total.
                    wps = psum_pool.tile([P, QW], mybir.dt.float32, name="ps_h", bufs=2)
                    nc.tensor.matmul(
                        wps[:, :],
                        warm_sb[:, :, :P],
                        warm_sb[:, :, :],
                        start=True,
                        stop=True,
                        perf_mode=mybir.MatmulPerfMode.DoubleRow,
                    )

            # Head DMA order (all on the Sync queue — secondary engine
            # queues get far worse DMA service): chunk0 (small) first, then
            # the two featT halves.  The first matmul group needs only
            # chunk0 + featT half 0.
            bank_sb0 = bank_pool.tile([P, 2 * KK, CHUNKS[0]], mybir.dt.float8e4)
            nc.sync.dma_start(bank_sb0[:, :, :], bank8[:, :, 0:CHUNKS[0]])

            # feature^T resident as two query-half tiles [128, kk, 2, QW] fp8
            # (4 KiB/part each); separate tiles so the qh=0 matmuls depend
            # only on the first half's DMA.  The middle "2" dim is the
            # DoubleRow K-pair (two 128-row planes of D).
            featT_h = []
            for h in range(2):
                f = feat_pool.tile([P, KK, 2, QW], mybir.dt.float8e4, name=f"featT_h{h}")
                nc.sync.dma_start(f[:, :, :, :], featT8[:, h, :, :, :])
                featT_h.append(f)

            def mm_group(ps, bank_sb, si, sw, qh):
                for kk in range(KK):
                    nc.tensor.matmul(
                        ps[:sw, qh * QW:(qh + 1) * QW],
                        bank_sb[:, 2 * kk:2 * kk + 2, si:si + sw],
                        featT_h[qh][:, kk, :, :],
                        start=(kk == 0),
                        stop=(kk == KK - 1),
                        perf_mode=mybir.MatmulPerfMode.DoubleRow,
                    )

            def drain(ps, val_t, sw, c0, si):
                # (sim - 64) clamped at 0, cast to u8 — single DVE op.
                nc.vector.tensor_scalar(
                    out=val_t[:sw, :],
                    in0=ps[:sw, :],
                    scalar1=-VAL_OFF,
                    scalar2=0.0,
                    op0=mybir.AluOpType.add,
                    op1=mybir.AluOpType.max,
                )
                nc.sync.dma_start(vals[c0 + si:c0 + si + sw, :], val_t[:sw, :])

            c0 = 0
            for ci, cw in enumerate(CHUNKS):
                last = ci == len(CHUNKS) - 1
                if ci == 0:
                    bank_sb = bank_sb0
                else:
                    bank_sb = bank_pool.tile([P, 2 * KK, cw], mybir.dt.float8e4, name="bank_sb")
                    nc.sync.dma_start(bank_sb[:, :, :], bank8[:, :, c0:c0 + cw])
                for si in range(0, cw, P):
                    sw = min(P, cw - si)
                    ps = psum_pool.tile([P, B], mybir.dt.float32, name="ps")
                    for qh in range(B // QW):
                        mm_group(ps, bank_sb, si, sw, qh)
                    if last and si + P >= cw:
                        # Final slice: drain and store per query-half so the
                        # tail after the very last matmul is short.
                        for qh in range(B // QW):
                            val_h = val_pool.tile([P, QW], mybir.dt.uint8)
                            nc.vector.tensor_scalar(
                                out=val_h[:sw, :],
                                in0=ps[:sw, qh * QW:(qh + 1) * QW],
                                scalar1=-VAL_OFF,
                                scalar2=0.0,
                                op0=mybir.AluOpType.add,
                                op1=mybir.AluOpType.max,
                            )
                            nc.sync.dma_start(
                                vals[c0 + si:c0 + si + sw, qh * QW:(qh + 1) * QW],
                                val_h[:sw, :],
                            )
                    else:
                        val_t = val_pool.tile([P, B], mybir.dt.uint8)
                        drain(ps, val_t, sw, c0, si)
                c0 += cw

    nc.compile()
    return nc


_PROGRAM_CACHE = {}


def _get_program(impl, n_shard):
    key = (impl, n_shard)
    if key not in _PROGRAM_CACHE:
        build = _build_program_fp8 if impl == "fp8" else _build_program_bf16
        _PROGRAM_CACHE[key] = build(n_shard)
    return _PROGRAM_CACHE[key]


def _profile_ctx():
    import contextlib

    @contextlib.contextmanager
    def _maybe_profile():
        """Optional NTFF capture via the axon NRT-profile C ABI."""
        prof_dir = os.environ.get("KNN_PROFILE_DIR")
        if not prof_dir:
            yield
            return
        import ctypes
        lib = ctypes.CDLL("/opt/axon/libaxon_pjrt.so")
        lib.axon_start_nrt_profile.argtypes = [
            ctypes.POINTER(ctypes.c_int64), ctypes.c_size_t]
        lib.axon_start_nrt_profile.restype = ctypes.c_int64
        lib.axon_stop_nrt_profile.argtypes = [ctypes.c_char_p]
        lib.axon_stop_nrt_profile.restype = ctypes.c_int64
        import jax
        jax.devices()
        rc = lib.axon_start_nrt_profile(None, 0)
        if rc != 0:
            raise RuntimeError(f"axon_start_nrt_profile rc={rc}")
        try:
            yield
        finally:
            n = lib.axon_stop_nrt_profile(str(prof_dir).encode())
            print(f"ntff profile: {n} file(s) -> {prof_dir}", flush=True)

    return _maybe_profile()


def _run_spmd(nc, in_maps):
    global LAST_EXEC_TIME_NS
    with _profile_ctx():
        res = run_bass_kernel_spmd(
            nc, in_maps, core_ids=list(range(N_CORES)), trace=False
        )
    LAST_EXEC_TIME_NS = res.exec_time_ns
    _tlog("device run done")
    return res


def _candidate_pairs_bf16(feature, bank_f32):
    """bf16+mask path: device mask -> all candidate pairs."""
    n = bank_f32.shape[1]
    n_shard = n // N_CORES
    nc = _get_program("bf16", n_shard)
    _tlog("program built")

    featT_bf = np.ascontiguousarray(feature.T).astype(ml_dtypes.bfloat16)
    bank_bf = bank_f32.astype(ml_dtypes.bfloat16)
    in_maps = [
        {
            "featT": featT_bf,
            "bank": np.ascontiguousarray(bank_bf[:, i * n_shard:(i + 1) * n_shard]),
        }
        for i in range(N_CORES)
    ]
    res = _run_spmd(nc, in_maps)
    mask = np.concatenate([res.results[i]["mask"] for i in range(N_CORES)], axis=0)

    nidx, qidx = np.nonzero(mask)  # [N, B]: sorted by bank idx
    order = np.argsort(qidx, kind="stable")  # per-query segments, nidx ascending
    qidx = qidx[order]
    nidx = nidx[order]
    counts = np.bincount(qidx, minlength=feature.shape[0])
    starts = np.zeros(feature.shape[0] + 1, dtype=np.int64)
    np.cumsum(counts, out=starts[1:])
    _tlog(f"candidates built ({len(nidx)} pairs)")
    return qidx, nidx, starts


def _candidate_pairs_fp8(feature, bank_f32, k):
    """fp8+values path: threshold, then keep only the top-k confidence window."""
    global LAST_DEV_VALS
    n = bank_f32.shape[1]
    n_shard = n // N_CORES
    nc = _get_program("fp8", n_shard)
    _tlog("program built")

    # Pre-tile to the device layouts (see _build_program_fp8 docstring).
    featT_8 = np.ascontiguousarray(feature.T).astype(ml_dtypes.float8_e4m3)
    featT_t = (
        featT_8.reshape(KK, 2, P, B).transpose(2, 0, 1, 3)
    )  # [128, KK, 2, B]; plane (kk,i) holds rows (2kk+i)*128+p
    featT_t = np.ascontiguousarray(
        featT_t.reshape(P, KK, 2, 2, QW).transpose(0, 3, 1, 2, 4)
    )  # [128, 2(q-half), KK, 2, QW] — each query-half contiguous
    bank_8 = bank_f32.astype(ml_dtypes.float8_e4m3)
    bank_t = bank_8.reshape(2 * KK, P, n).transpose(1, 0, 2)  # [128, 8, n]
    pad = np.zeros((P, 2 * KK, N_PAD - n_shard), dtype=ml_dtypes.float8_e4m3)
    in_maps = [
        {
            "featT8": featT_t,
            "bank8": np.ascontiguousarray(
                np.concatenate(
                    [bank_t[:, :, i * n_shard:(i + 1) * n_shard], pad], axis=2
                )
            ),
        }
        for i in range(N_CORES)
    ]
    res = _run_spmd(nc, in_maps)
    vals = np.concatenate(
        [res.results[i]["vals"][:n_shard] for i in range(N_CORES)], axis=0
    )
    LAST_DEV_VALS = vals  # [N, B] u8: clamp(sim - VAL_OFF, 0, 255)

    m = vals >= np.uint8(T0_FP8_U8)
    nidx, qidx = np.nonzero(m)
    # The DVE f32->u8 cast rounds-to-nearest, so stored+VAL_OFF is already the
    # quantization-interval midpoint (E_FP8 covers the +-0.5 either way).
    dv = vals[nidx, qidx].astype(np.float32) + np.float32(VAL_OFF)
    order = np.argsort(qidx, kind="stable")  # per-query segments, nidx ascending
    qidx = qidx[order]
    nidx = nidx[order]
    dv = dv[order]
    b = feature.shape[0]
    counts = np.bincount(qidx, minlength=b)
    starts_all = np.zeros(b + 1, dtype=np.int64)
    np.cumsum(counts, out=starts_all[1:])
    _tlog(f"thresholded ({len(nidx)} pairs)")

    # Per query, keep only candidates that can possibly be in the true top-k:
    # dev >= dev_rank_k - 2E (see module docstring for the bound).
    keep = np.zeros(len(nidx), dtype=bool)
    for q in range(b):
        s, e = starts_all[q], starts_all[q + 1]
        c = e - s
        if c < k:
            keep[s:e] = True  # top-k loop will take the full-row fallback
            continue
        seg = dv[s:e]
        rk = np.partition(seg, c - k)[c - k]
        keep[s:e] = seg >= rk - 2.0 * E_FP8
    qidx = qidx[keep]
    nidx = nidx[keep]
    counts = np.bincount(qidx, minlength=b)
    starts = np.zeros(b + 1, dtype=np.int64)
    np.cumsum(counts, out=starts[1:])
    _tlog(f"windowed ({len(nidx)} pairs)")
    return qidx, nidx, starts


def _finish(feature, bank_f32, labels, num_classes, k, cand):
    """Exact fp32 re-rank of candidate pairs + reference post-processing.

    cand is (qidx, nidx, starts) or None (full host fallback).
    """
    b, d = feature.shape
    n = bank_f32.shape[1]

    if cand is not None:
        qidx, nidx, starts = cand
        bankT = np.ascontiguousarray(bank_f32.T)  # contiguous row gathers
        _tlog("bankT transpose done")
        vals = np.empty(len(nidx), dtype=np.float32)
        CHP = 1 << 16
        for s in range(0, len(nidx), CHP):
            e = min(s + CHP, len(nidx))
            vals[s:e] = np.einsum(
                "ij,ij->i", feature[qidx[s:e]], bankT[nidx[s:e]]
            )
        _tlog(f"exact vals done ({len(nidx)} pairs)")

    full_rows = None
    full_q0 = 0
    all_idx = np.arange(n)

    sel_q = np.empty(b * k, dtype=np.int64)
    sel_lab = np.empty(b * k, dtype=np.int64)
    sel_val = np.empty(b * k, dtype=np.float32)
    pos = 0
    ROWBLK = 64
    for q in range(b):
        if cand is not None and starts[q + 1] - starts[q] >= k:
            s, e = starts[q], starts[q + 1]
            v = vals[s:e]
            idx = nidx[s:e]
        else:
            # Exact full row (no device pre-filter, or threshold miss).
            if full_rows is None or not (full_q0 <= q < full_q0 + ROWBLK):
                full_q0 = q
                hi = min(q + ROWBLK, b)
                full_rows = feature[q:hi] @ bank_f32
            v = full_rows[q - full_q0]
            idx = all_idx
        # jax.lax.top_k semantics: descending, ties -> lower index first.
        order = np.argsort(-v, kind="stable")[:k]
        sel_q[pos:pos + k] = q
        sel_lab[pos:pos + k] = labels[idx[order]]
        sel_val[pos:pos + k] = v[order]
        pos += k
    _tlog("per-query topk done")

    with np.errstate(over="ignore"):
        w = np.exp(sel_val / np.float32(KNN_T)).astype(np.float32)
    scores = np.zeros((b, num_classes), dtype=np.float32)
    np.add.at(scores, (sel_q, sel_lab), w)
    _tlog("scatter done")
    return scores


def kernel(feature, feature_bank, feature_labels, num_classes, knn_k):
    _tlog("kernel() start")
    feature = np.asarray(feature, dtype=np.float32)
    bank_f32 = np.asarray(feature_bank, dtype=np.float32)
    labels = np.asarray(feature_labels)
    c = int(np.asarray(num_classes))
    k = int(np.asarray(knn_k))

    b, d = feature.shape
    n = bank_f32.shape[1]

    impl = os.environ.get("KNN_IMPL", "fp8")
    use_device = d == D and b == B and n % N_CORES == 0 and n // N_CORES > 0
    if use_device:
        if impl == "fp8":
            cand = _candidate_pairs_fp8(feature, bank_f32, k)
        else:
            cand = _candidate_pairs_bf16(feature, bank_f32)
    else:
        cand = None  # degenerate fallback: host does it all

    scores = _finish(feature, bank_f32, labels, c, k, cand)
    pred = np.argsort(-scores, axis=1, kind="stable").astype(np.int32)
    _tlog("final argsort done")
    return pred

